# revision 1
# baseline (speedup 1.0000x reference)
# Multi-head attention (B=2, S=2048, D=1024, H=16, head_dim=64) with bool mask,
# sharded across 8 TRN2 NeuronCores: core c -> batch c//4, heads 4*(c%4)..4*(c%4)+3.
#
# Per-core device kernel (scores computed transposed: scoresT[k, q]):
#   scoresT = K @ Q^T                (PE bf16, lhsT = K^T chunk, rhs = Q^T)
#   attnT   = exp(scoresT/8) * (1-m)T (ACT exp with scale=1/8 -> bf16, DVE mult)
#   outT'   = [V | 1]^T @ attnT      (PE bf16; row 64 = softmax denominator Z)
#   out     = transpose(outT') / Z   (PE transpose + batched DVE normalize)
#
# Host side (inside kernel()): slice per-core shards, pre-transpose Q/K per head
# ([64, S] head-dim-major, bf16), pre-transpose the inverted mask to bf16,
# reassemble the 8 per-core bf16 outputs into the full f32 [B, S, D] output.

import sys

import numpy as np

for _p in ("/opt/trn_rl_repo",):
    if _p not in sys.path:
        sys.path.insert(0, _p)

import ml_dtypes

import concourse.bass as bass  # noqa: F401  (engine types reachable via nc)
import concourse.tile as tile
from concourse import bacc, mybir
from concourse.bass_utils import run_bass_kernel_spmd
from concourse.masks import make_identity

F32 = mybir.dt.float32
F32R = mybir.dt.float32r
BF16 = mybir.dt.bfloat16

S = 2048          # sequence length
HD = 64           # head dim
HPC = 4           # heads per core
NCORES = 8
B = 2
H = 16
D = H * HD


def build_program(s=S, act_dtype=BF16, qk_dtype=BF16, n_psS=2, reps=1):
    """Build the single-core SPMD program. Returns the compiled Bacc object.

    reps>1 emits the whole body (loads+compute+stores) that many times in one
    NEFF — used to measure device time by wall-clock differencing."""
    nc = bacc.Bacc()

    KS = s // 128            # number of k strips
    QG = 1024 if s >= 1024 else s   # q group width (ACT/DVE instruction width)
    NQG = s // QG            # q groups
    NQC = max(QG // 512, 1)  # 512-wide matmul chunks per q group
    QC = min(512, QG)        # matmul chunk width
    JT = QG // 128           # out-transpose chunks per q group

    qkT_d = nc.declare_dram_parameter("qkT", [2, HPC * HD, s], qk_dtype, isOutput=False)
    v_d = nc.declare_dram_parameter("v", [s, HPC * HD], BF16, isOutput=False)
    nmT_d = nc.declare_dram_parameter("nmT", [s, s], BF16, isOutput=False)
    out_d = nc.declare_dram_parameter("out", [s, HPC * HD], BF16, isOutput=True)

    # DRAM views with the k/q axis split into strips of 128 partitions
    nm_view = nmT_d[:].rearrange("(ks p) q -> p ks q", p=128)
    v_view = v_d[:].rearrange("(ks p) c -> p ks c", p=128)
    out_view = out_d[:].rearrange("(sq p) c -> p sq c", p=128)

    with tile.TileContext(nc) as tc:
        with (
            tc.tile_pool(name="const", bufs=1) as const,
            tc.tile_pool(name="wq", bufs=1) as wq,
            tc.tile_pool(name="vstg", bufs=1) as vstg,
            tc.tile_pool(name="attn", bufs=20) as apool,
            tc.tile_pool(name="fin", bufs=2) as fpool,
            tc.tile_pool(name="stat", bufs=4) as spool,
            tc.tile_pool(name="oasm", bufs=1) as opool,
            tc.tile_pool(name="psS", bufs=n_psS, space="PSUM") as psS_pool,
            # psO (AV accumulator, [65,QG]=2 banks) and pn (out-transpose
            # target, [128,JT,128]=2 banks) share one tag with bufs=2: the
            # two slots alternate psO/pn roles, so AV of group g only waits
            # for the finalize reads of group g-2 (1.5 groups of slack).
            tc.tile_pool(name="psF", bufs=2, space="PSUM") as psF_pool,
        ):
            ident = const.tile([128, 128], F32)
            make_identity(nc, ident)

            # Preload the exp table (emitted before any real exp; runs while
            # the first DMAs stream).
            warm = const.tile([128, 1], F32)
            nc.vector.memset(warm, 0.0)
            nc.scalar.activation(warm, warm, mybir.ActivationFunctionType.Exp)

            # Warm the PE HAM clock gate while input DMAs run: ~3us of dummy
            # matmuls (transpose-mode doesn't count as PE-busy for HAM) so
            # the first real QKs run at 2.4GHz.
            zb = const.tile([128, 128], BF16)
            nc.vector.memset(zb, 0.0)
            for _ in range(24):
                wmm = psS_pool.tile([128, QG], F32, tag="psS")
                nc.tensor.matmul(
                    wmm[:, :128], lhsT=zb[0:64, :], rhs=zb[0:64, :],
                    start=True, stop=True,
                )

            def qk_src(pair):
                return qkT_d[:, 128 * pair:128 * pair + 128, :].rearrange(
                    "t p s -> p t s"
                )

            def emit_body():
                # Q^T / K^T head pairs: [128, s] (head 2p on partitions 0-63,
                # head 2p+1 on partitions 64-127). The first pair's q and k
                # halves ride different HWDGE queues in parallel; everything
                # else is emitted in the order compute consumes it.
                qks = []
                for pair in range(HPC // 2):
                    qk = wq.tile([128, 2, s], qk_dtype, tag=f"qkT{pair}")
                    qks.append(qk)
                v_sb = vstg.tile([128, KS, HPC * HD], BF16)
                nm_sb = wq.tile([128, KS, s], BF16, tag="nm")
                KH = KS // 2
                nc.scalar.dma_start(out=qks[0][:, 0, :], in_=qk_src(0)[:, 0, :])
                nc.sync.dma_start(out=qks[0][:, 1, :], in_=qk_src(0)[:, 1, :])
                nc.sync.dma_start(out=v_sb[:, :KH], in_=v_view[:, :KH])
                nc.sync.dma_start(out=v_sb[:, KH:], in_=v_view[:, KH:])
                for pair in range(1, HPC // 2):
                    nc.scalar.dma_start(out=qks[pair], in_=qk_src(pair))
                for ks in range(KS):
                    nc.sync.dma_start(out=nm_sb[:, ks, :], in_=nm_view[:, ks, :])

                # V' = [V | 1] per head, bf16; cast in halves so early AVs
                # only wait on the first half of the V DMA.
                vps = []
                for h in range(HPC):
                    vp = wq.tile([128, KS, HD + 1], BF16, tag=f"vp{h}")
                    vps.append(vp)
                for half in range(2):
                    ksl = slice(half * KH, KH + half * KH)
                    for h in range(HPC):
                        nc.vector.tensor_copy(
                            out=vps[h][:, ksl, 0:HD],
                            in_=v_sb[:, ksl, h * HD:(h + 1) * HD],
                        )
                        nc.vector.memset(vps[h][:, ksl, HD:HD + 1], 1.0)

                out_asm = opool.tile([128, KS, HPC * HD], BF16)

                # Emission state threading three overlapped group pipelines:
                #   carry - group awaiting last AV (stop=True) + psO->oT copy
                #   fin   - group awaiting its transpose+normalize steps
                fin = {"pend": None, "idx": 0, "pn": None}
                N_FIN = JT + 1  # JT transposes + one batched normalize step

                def finalize_step():
                    """One finalize chunk of a finished q-group: steps
                    0..JT-1 transpose [65,128] pieces into pn; step JT does
                    one strided reciprocal over the JT Z values and two
                    broadcast multiplies (batched - avoids per-strip
                    sequencer overhead)."""
                    h, qg, oT = fin["pend"]
                    if fin["idx"] >= N_FIN:
                        return
                    j = fin["idx"]
                    fin["idx"] += 1
                    if j == 0:
                        pn_t = psF_pool.tile([128, JT, 128], F32, tag="fin")
                        fin["pn"] = pn_t
                    pn = fin["pn"]
                    if j < JT:
                        nc.tensor.transpose(
                            pn[:, j, :HD + 1],
                            oT[:, j * 128:(j + 1) * 128],
                            ident[:HD + 1, :HD + 1],
                        )
                        return
                    rec8 = spool.tile([128, JT], F32)
                    nc.vector.reciprocal(rec8, pn[:, :, HD])
                    half = (JT + 1) // 2
                    for lo in range(0, JT, half):
                        hi = min(lo + half, JT)
                        sq0 = qg * JT + lo
                        nc.vector.tensor_mul(
                            out_asm[:, sq0:sq0 + hi - lo, h * HD:(h + 1) * HD],
                            pn[:, lo:hi, 0:HD],
                            rec8[:, lo:hi].to_broadcast([128, hi - lo, HD]),
                        )
                        if h == HPC - 1:
                            eng = nc.sync if lo == 0 else nc.scalar
                            eng.dma_start(
                                out=out_view[:, sq0:sq0 + hi - lo, :],
                                in_=out_asm[:, sq0:sq0 + hi - lo, :],
                            )

                def emit_carry(carry):
                    """Last AV (stop=True) + psO->SBUF copy of a group."""
                    ch, cqg, cpsO, cat = carry
                    for qc in range(NQC):
                        nc.tensor.matmul(
                            cpsO[:, qc * QC:(qc + 1) * QC],
                            lhsT=vps[ch][:, KS - 1, :],
                            rhs=cat[:, qc * QC:(qc + 1) * QC],
                            start=(KS == 1),
                            stop=True,
                        )
                    oT = fpool.tile([HD + 1, QG], F32, tag="oT")
                    nc.vector.tensor_copy(oT, cpsO)
                    # flush unfinished finalize steps of the older group
                    while fin["pend"] is not None and fin["idx"] < N_FIN:
                        finalize_step()
                    fin["pend"] = (ch, cqg, oT)
                    fin["idx"] = 0

                carry = None
                groups = [(h, qg) for h in range(HPC) for qg in range(NQG)]
                for h, qg in groups:
                    base = 64 * (h % 2)
                    qt_r = qks[h // 2][:, 0, :]
                    kt_r = qks[h // 2][:, 1, :]
                    q0 = qg * QG
                    psO = None
                    at_prev = None
                    for ks in range(KS):
                        # AV one strip behind QK, emitted BEFORE this strip's
                        # QK so it isn't queued behind QK's psum-slot wait.
                        if at_prev is not None:
                            if psO is None:
                                psO = psF_pool.tile(
                                    [HD + 1, QG], F32, tag="fin"
                                )
                            for qc in range(NQC):
                                nc.tensor.matmul(
                                    psO[:, qc * QC:(qc + 1) * QC],
                                    lhsT=vps[h][:, ks - 1, :],
                                    rhs=at_prev[:, qc * QC:(qc + 1) * QC],
                                    start=(ks == 1),
                                    stop=False,
                                )
                        # Transpose+normalize of an older group, interleaved
                        # so it never stalls the PE pipeline.
                        if fin["pend"] is not None and ks >= 1:
                            finalize_step()
                        psS = psS_pool.tile([128, QG], F32)
                        for qc in range(NQC):
                            nc.tensor.matmul(
                                psS[:, qc * QC:(qc + 1) * QC],
                                lhsT=kt_r[base:base + HD, ks * 128:(ks + 1) * 128],
                                rhs=qt_r[base:base + HD,
                                         q0 + qc * QC:q0 + (qc + 1) * QC],
                                start=True,
                                stop=True,
                            )
                        if ks == 0 and carry is not None:
                            emit_carry(carry)
                            carry = None
                        at = apool.tile([128, QG], act_dtype, tag="at")
                        nc.scalar.activation(
                            at, psS, mybir.ActivationFunctionType.Exp,
                            scale=0.125,
                        )
                        nc.vector.tensor_mul(at, at, nm_sb[:, ks, q0:q0 + QG])
                        at_prev = at
                    carry = (h, qg, psO, at_prev)
                emit_carry(carry)
                while fin["idx"] < N_FIN:
                    finalize_step()

            for _ in range(reps):
                emit_body()
    nc.compile()
    return nc


_CACHE = {}


def _get_nc():
    if "nc" not in _CACHE:
        _CACHE["nc"] = build_program()
    return _CACHE["nc"]


def make_in_maps(q, k, v, mask, s=S):
    """Shard full inputs into 8 per-core input maps (host-side layout prep)."""
    q = np.asarray(q, dtype=np.float32)
    k = np.asarray(k, dtype=np.float32)
    v = np.asarray(v, dtype=np.float32)
    mask = np.asarray(mask)
    nh = q.shape[-1] // HD
    in_maps = []
    for c in range(NCORES):
        b, g = divmod(c, NCORES // B)
        h0 = HPC * g
        qs = q[b].reshape(s, nh, HD)[:, h0:h0 + HPC, :]      # [s, HPC, 64]
        ks_ = k[b].reshape(s, nh, HD)[:, h0:h0 + HPC, :]
        qkT = np.empty((2, HPC * HD, s), ml_dtypes.bfloat16)
        qkT[0] = qs.transpose(1, 2, 0).reshape(HPC * HD, s)
        qkT[1] = ks_.transpose(1, 2, 0).reshape(HPC * HD, s)
        vc = np.ascontiguousarray(v[b, :, h0 * HD:(h0 + HPC) * HD]).astype(
            ml_dtypes.bfloat16
        )
        nmT = np.ascontiguousarray((~mask[b]).T).astype(ml_dtypes.bfloat16)
        in_maps.append({"qkT": qkT, "v": vc, "nmT": nmT})
    return in_maps


def assemble_out(results, s=S, d=D):
    out = np.empty((B, s, d), np.float32)
    for c in range(NCORES):
        b, g = divmod(c, NCORES // B)
        out[b, :, g * HPC * HD:(g + 1) * HPC * HD] = results[c]["out"]
    return out


def kernel(q, k, v, mask):
    nc = _get_nc()
    in_maps = make_in_maps(q, k, v, mask)
    res = run_bass_kernel_spmd(nc, in_maps, list(range(NCORES))).results
    return assemble_out(res)



# revision 25
# speedup vs baseline: 1.4397x; 1.4397x over previous
# Multi-head attention (B=2, S=2048, D=1024, H=16, head_dim=64) with bool mask,
# sharded across 8 TRN2 NeuronCores: core c -> batch c//4, heads 4*(c%4)..4*(c%4)+3.
#
# Per-core device kernel:
#   scoresT = K @ Q^T                 (PE bf16, [128 k, 1024 q] units)
#   eviction of each psS unit to bf16 attn, split 3 ways to balance engines:
#     'A': ACT exp(scale=1/8) -> DVE mask multiply
#     'P': ACT exp(scale=1/8) -> Pool (gpsimd) mask multiply
#     'Z': one fused DVE scalar_tensor_tensor: i16 <- (psS + B') * m'[k,q],
#          bit-reinterpreted as bf16 == Schraudolph exp(s/8) with the mask
#          folded in. The mask tile holds {A'=23.125, 0}; on the A/P paths the
#          same tile is a plain multiplicative mask whose uniform A' factor
#          cancels in the softmax normalization. B' is tuned so the Z path's
#          mean scale matches the A/P paths' A'*exp(s/8) exactly.
#   AV in direct layout: out[q,d] = attnT^T @ [V|1] per 128-q chunk (PE bf16,
#   full 128 output partitions; column 64 is the softmax denominator Z).
#   normalize: DVE reciprocal + broadcast multiply, assembled in SBUF, DMA out.
#
# Host side (inside kernel()): slice per-core shards, pre-transpose Q/K per head
# ([64, S] head-dim-major, bf16), pre-bake the inverted mask transposed as
# {A', 0} bf16, reassemble the 8 per-core bf16 outputs into [B, S, D] f32.

import sys

import numpy as np

for _p in ("/opt/trn_rl_repo",):
    if _p not in sys.path:
        sys.path.insert(0, _p)

import ml_dtypes

import concourse.bass as bass  # noqa: F401  (engine types reachable via nc)
import concourse.tile as tile
from concourse import bacc, mybir

F32 = mybir.dt.float32
BF16 = mybir.dt.bfloat16
I16 = mybir.dt.int16

S = 2048          # sequence length
HD = 64           # head dim
HPC = 4           # heads per core
NCORES = 8
B = 2
H = 16
D = H * HD

# Schraudolph constants for the Z path. A' is the exact bf16 rounding of
# 128/(8*ln2); B' is tuned (float32, truncating i16 cast) so that
# E[bitcast_bf16(i16((s+B')*A'))] == A' * exp(s/8) over the score distribution.
A_PRIME = 23.125
B_PRIME = 727.746979

# Optional debug map: instruction name -> semantic label (filled when
# DEBUG_LABELS is a dict; costs nothing when None).
DEBUG_LABELS = None


def _dbg(ins, label):
    if DEBUG_LABELS is not None and ins is not None:
        try:
            DEBUG_LABELS[ins.ins.name] = label
        except AttributeError:
            pass

# Per-phase eviction path patterns (16 k-strip units per phase), alternating.
# Z = fused DVE bit-trick, A = ACT exp + DVE mask, P = ACT exp + Pool mask.
# Pool mask-multiplies are the slowest (~2.1us), and the next phase's AV
# matmuls read every strip of this phase - so P units never occupy the last
# three units of a phase (a laggy Pool TT there stalls the in-order PE).
PATTERNS = ["PAZPAZPAZPAZPZAZ", "PAZPAZPAZPAZPAZA"]


def build_program(s=S, reps=1, patterns=PATTERNS):
    """Build the single-core SPMD program. Returns the compiled Bacc object.

    reps>1 emits the whole body that many times in one NEFF - used to measure
    device time by wall-clock differencing."""
    nc = bacc.Bacc()

    KS = s // 128            # number of k strips
    QG = min(1024, s)        # q width of one eviction unit
    NQG = s // QG            # q groups ("halves" at s=2048)
    NCH = QG // 128          # AV q-chunks per group
    CPG = min(4, NCH)        # chunks per psO group

    qkT_d = nc.declare_dram_parameter("qkT", [2, HPC * HD, s], BF16, isOutput=False)
    v_d = nc.declare_dram_parameter("v", [s, HPC * HD], BF16, isOutput=False)
    nmT_d = nc.declare_dram_parameter("nmT", [s, s], BF16, isOutput=False)
    out_d = nc.declare_dram_parameter("out", [s, HPC * HD], BF16, isOutput=True)

    nm_view = nmT_d[:].rearrange("(ks p) q -> p ks q", p=128)
    v_view = v_d[:].rearrange("(ks p) c -> p ks c", p=128)
    out_view = out_d[:].rearrange("(sq p) c -> p sq c", p=128)

    with tile.TileContext(nc) as tc:
        with (
            tc.tile_pool(name="const", bufs=1) as const,
            tc.tile_pool(name="wq", bufs=1) as wq,
            tc.tile_pool(name="attn", bufs=min(2 * KS + 4, 36)) as apool,
            tc.tile_pool(name="stat", bufs=4) as spool,
            tc.tile_pool(name="oasm", bufs=1) as opool,
            tc.tile_pool(name="psS", bufs=3, space="PSUM") as psS_pool,
            tc.tile_pool(name="psO", bufs=2, space="PSUM") as psO_pool,
        ):
            # Preload the exp table (emitted before any real exp; runs while
            # the first DMAs stream).
            warm = const.tile([128, 1], F32)
            nc.vector.memset(warm, 0.0)
            nc.scalar.activation(warm, warm, mybir.ActivationFunctionType.Exp)

            # Warm the PE clock (cost model p-state ramp) while input DMAs
            # stream: ~3us of dummy matmuls.
            zb = const.tile([128, 128], BF16)
            nc.vector.memset(zb, 0.0)
            for _ in range(30):
                wmm = psS_pool.tile([128, QG], F32, tag="psS")
                nc.tensor.matmul(
                    wmm[:, :128], lhsT=zb[0:64, :], rhs=zb[0:64, :],
                    start=True, stop=True,
                )

            def qk_src(pair):
                return qkT_d[:, 128 * pair:128 * pair + 128, :].rearrange(
                    "t p s -> p t s"
                )

            def emit_body():
                # Q^T / K^T head pairs: [128, 2, s] (head 2p on partitions
                # 0-63, head 2p+1 on 64-127; dim1: 0=Q^T, 1=K^T).
                qks = []
                for pair in range(HPC // 2):
                    qk = wq.tile([128, 2, s], BF16, tag=f"qkT{pair}")
                    qks.append(qk)
                v_sb = wq.tile([128, KS, HPC * HD], BF16, tag="vsb")
                nm_sb = wq.tile([128, KS, s], BF16, tag="nm")
                KH = KS // 2
                # All input DMAs ride the SP HWDGE queue (SP has no compute,
                # so ring-full stalls never block a compute sequencer; gpsimd
                # dma_start is SWDGE and would burn Pool engine time). Pieces
                # are ordered by first use; phases run q-group-major, so mask
                # q-group 1 is not needed until ~halfway through the kernel.
                def nm_piece(ks, g):
                    nc.sync.dma_start(
                        out=nm_sb[:, ks, g * QG:(g + 1) * QG],
                        in_=nm_view[:, ks, g * QG:(g + 1) * QG],
                    )

                # First Q/K pair split by head (partition halves) so head 0's
                # slices land in ~a quarter of the full-pair DMA time.
                nc.scalar.dma_start(
                    out=qks[0][0:HD, 0, :], in_=qk_src(0)[0:HD, 0, :]
                )
                nc.sync.dma_start(
                    out=qks[0][0:HD, 1, :], in_=qk_src(0)[0:HD, 1, :]
                )
                nc.scalar.dma_start(
                    out=qks[0][HD:, 0, :], in_=qk_src(0)[HD:, 0, :]
                )
                nc.sync.dma_start(
                    out=qks[0][HD:, 1, :], in_=qk_src(0)[HD:, 1, :]
                )
                for ks in range(KS):
                    nm_piece(ks, 0)
                nc.sync.dma_start(out=v_sb[:, :KH], in_=v_view[:, :KH])
                nc.sync.dma_start(out=v_sb[:, KH:], in_=v_view[:, KH:])
                for pair in range(1, HPC // 2):
                    nc.sync.dma_start(out=qks[pair], in_=qk_src(pair))
                for g in range(1, NQG):
                    for ks in range(KS):
                        nm_piece(ks, g)

                # V' = [V | 1] per head, bf16 (column 64 computes the softmax
                # denominator in the AV matmul). Copy in halves so early AV
                # only waits on the first half of the V DMA.
                vps = []
                for h in range(HPC):
                    vp = wq.tile([128, KS, HD + 1], BF16, tag=f"vp{h}")
                    vps.append(vp)
                for half in range(2):
                    ksl = slice(half * KH, KH + half * KH)
                    for h in range(HPC):
                        nc.vector.tensor_copy(
                            out=vps[h][:, ksl, 0:HD],
                            in_=v_sb[:, ksl, h * HD:(h + 1) * HD],
                        )
                        nc.vector.memset(vps[h][:, ksl, HD:HD + 1], 1.0)

                out_asm = opool.tile([128, KS, HPC * HD], BF16)

                # q-group-major phase order: the first HPC phases only touch
                # mask q-group 0, giving the mask DMA stream headroom.
                phases = [(h, g) for g in range(NQG) for h in range(HPC)]

                def emit_av_chunk(ph, c, av_state):
                    """AV matmuls for q-chunk c of phase ph, plus group
                    finalize (reciprocal + normalize) every CPG chunks.

                    Strips are read in eviction-completion order (Z first,
                    then A, then P): the last strips read are the ones whose
                    masks lag past the phase boundary, so the PE never waits
                    on a straggling Pool/DVE mask with work still in hand."""
                    h, g = ph
                    ats = av_state["ats"]
                    order = av_state["order"]
                    if c % CPG == 0:
                        av_state["psO"] = psO_pool.tile(
                            [128, CPG, 128], F32, tag="psO", name="psO"
                        )
                    psO = av_state["psO"]
                    for i, ks in enumerate(order):
                        _dbg(nc.tensor.matmul(
                            psO[:, c % CPG, 0:HD + 1],
                            lhsT=ats[ks][:, c * 128:(c + 1) * 128],
                            rhs=vps[h][:, ks, :],
                            start=(i == 0),
                            stop=(i == KS - 1),
                        ), f"AV h{h}g{g} c{c} ks{ks}")
                    if c % CPG == CPG - 1:
                        c0 = c - (CPG - 1)
                        qc0 = g * NCH + c0
                        rec = spool.tile([128, CPG], F32, tag="rec")
                        _dbg(nc.vector.reciprocal(rec, psO[:, :, HD]),
                             f"recip h{h}g{g} c{c}")
                        _dbg(nc.vector.tensor_mul(
                            out_asm[:, qc0:qc0 + CPG, h * HD:(h + 1) * HD],
                            psO[:, :, 0:HD],
                            rec.to_broadcast([128, CPG, HD]),
                        ), f"norm h{h}g{g} c{c}")
                        if h == HPC - 1:
                            nc.sync.dma_start(
                                out=out_view[:, qc0:qc0 + CPG, :],
                                in_=out_asm[:, qc0:qc0 + CPG, :],
                            )

                prev = None  # (phase, {"ats": [...]}) awaiting AV
                unit = 0  # global eviction-unit counter (for path pattern)
                for ph in phases:
                    h, g = ph
                    base = HD * (h % 2)
                    pair = h // 2
                    q0 = g * QG
                    ats = []
                    paths = []
                    for ks in range(KS):
                        # QK for this unit
                        psS = psS_pool.tile([128, QG], F32, tag="psS")
                        for qc in range(QG // 512):
                            _dbg(nc.tensor.matmul(
                                psS[:, qc * 512:(qc + 1) * 512],
                                lhsT=qks[pair][base:base + HD, 1,
                                               ks * 128:(ks + 1) * 128],
                                rhs=qks[pair][base:base + HD, 0,
                                              q0 + qc * 512:q0 + (qc + 1) * 512],
                                start=True,
                                stop=True,
                            ), f"QK h{h}g{g} ks{ks}")
                        # Chunks ride units 4..~12: late enough that the
                        # previous phase's last evictions have drained, early
                        # enough that attn slots recycle before phase p+2.
                        if prev is not None:
                            start = 4 if KS > 8 else 1
                            den = max(KS - start - 2, 1)
                            for c in range(NCH):
                                if min(start + c * den // NCH, KS - 1) == ks:
                                    emit_av_chunk(prev[0], c, prev[1])
                        # Eviction: psS -> masked bf16 attn tile
                        at = apool.tile([128, QG], BF16, tag="at")
                        nm_slice = nm_sb[:, ks, q0:q0 + QG]
                        pat = patterns[(unit // KS) % len(patterns)]
                        path = pat[ks % len(pat)]
                        unit += 1
                        if path == "Z":
                            _dbg(nc.vector.scalar_tensor_tensor(
                                at[:].bitcast(I16),
                                psS[:],
                                B_PRIME,
                                nm_slice,
                                mybir.AluOpType.add,
                                mybir.AluOpType.mult,
                            ), f"STT h{h}g{g} ks{ks}")
                        else:
                            _dbg(nc.scalar.activation(
                                at, psS, mybir.ActivationFunctionType.Exp,
                                scale=0.125,
                            ), f"exp{path} h{h}g{g} ks{ks}")
                            if path == "A":
                                _dbg(nc.vector.tensor_mul(at, at, nm_slice),
                                     f"maskA h{h}g{g} ks{ks}")
                            else:
                                _dbg(nc.gpsimd.tensor_mul(at, at, nm_slice),
                                     f"maskP h{h}g{g} ks{ks}")
                        ats.append(at)
                        paths.append(path)
                    rank = {"Z": 0, "A": 1, "P": 2}
                    order = sorted(range(KS), key=lambda k: (rank[paths[k]], k))
                    prev = (ph, {"ats": ats, "order": order})
                # Tail: AV of the final phase
                for c in range(NCH):
                    emit_av_chunk(prev[0], c, prev[1])

            for _ in range(reps):
                emit_body()
    nc.compile()
    return nc


_CACHE = {}


def _get_nc():
    if "nc" not in _CACHE:
        _CACHE["nc"] = build_program()
    return _CACHE["nc"]


def make_in_maps(q, k, v, mask, s=S):
    """Shard full inputs into 8 per-core input maps (host-side layout prep)."""
    q = np.asarray(q, dtype=np.float32)
    k = np.asarray(k, dtype=np.float32)
    v = np.asarray(v, dtype=np.float32)
    mask = np.asarray(mask)
    nh = q.shape[-1] // HD
    in_maps = []
    for c in range(NCORES):
        b, g = divmod(c, NCORES // B)
        h0 = HPC * g
        qs = q[b].reshape(s, nh, HD)[:, h0:h0 + HPC, :]      # [s, HPC, 64]
        ks_ = k[b].reshape(s, nh, HD)[:, h0:h0 + HPC, :]
        qkT = np.empty((2, HPC * HD, s), ml_dtypes.bfloat16)
        qkT[0] = qs.transpose(1, 2, 0).reshape(HPC * HD, s)
        qkT[1] = ks_.transpose(1, 2, 0).reshape(HPC * HD, s)
        vc = np.ascontiguousarray(v[b, :, h0 * HD:(h0 + HPC) * HD]).astype(
            ml_dtypes.bfloat16
        )
        nmT = (np.float32(A_PRIME) * (~mask[b]).T.astype(np.float32)).astype(
            ml_dtypes.bfloat16
        )
        in_maps.append({"qkT": qkT, "v": vc, "nmT": nmT})
    return in_maps


def assemble_out(results, s=S, d=D):
    out = np.empty((B, s, d), np.float32)
    for c in range(NCORES):
        b, g = divmod(c, NCORES // B)
        out[b, :, g * HPC * HD:(g + 1) * HPC * HD] = results[c]["out"]
    return out


def kernel(q, k, v, mask):
    from concourse.bass_utils import run_bass_kernel_spmd

    nc = _get_nc()
    in_maps = make_in_maps(q, k, v, mask)
    res = run_bass_kernel_spmd(nc, in_maps, list(range(NCORES))).results
    return assemble_out(res)
